# revision 66
# baseline (speedup 1.0000x reference)
"""GraphSAGE layer kernel for Trainium2, SPMD over 8 NeuronCores.

Math (per reference):
    x3   = inputs.reshape(B, N, D)                      # B=128, N=4096, D=32
    out  = relu(x3 @ W_self + (A^T_contract @ (x3 @ W_neigh)))   # per batch
    out  = out.reshape(B, N*D)
with out[b, j, q] = relu(sum_p x3[b,j,p] Ws[p,q] + sum_i adj[i,j] (x3 @ Wn)[b,i,q])

Strategy:
  - Pure data-parallel over batch: 16 batches per core.
  - The big aggregation matmul runs in fp8 (e4m3) with DoubleRow perf mode:
    each matmul consumes a 256-deep contraction (2 k-tiles of 128) at half
    the per-row cost of fp16 -> 4x fewer PE cycles than the fp16 baseline.
    adj is host-prescaled by 2^12 so its entries (~1/N) sit in the fp8
    normal range; W_self is prescaled by 2^12 to match, and the host
    multiplies the output by 2^-12 after gather (relu commutes with
    positive scaling).
  - T = X @ Wn is computed on-device in fp16 (via a block-diagonal weight
    trick: 4 batches x 32 features on the 128 contraction lanes), then cast
    to fp8 during PSUM evacuation.  The self part X @ (2^12 Ws) is NOT
    materialized: it is accumulated directly into the aggregation PSUM with
    4 extra fp16 matmuls per output block (stationary = resident XT chunk).
  - Precision: X and Ws stay fp16 (the self term dominates the output).
    Only adj and T are fp8; they only feed the neighbor term whose
    magnitude is ~1/sqrt(N) of the self term, so fp8 error there is ~1e-3
    relative on the output, far below the 2e-2 gate.
  - DMA: A is fp8 (16 MB/core: half-panel pieces for the first three
    panels, then 1/2-panel sub-groups), XT fp16 in 16 two-block chunks
    (finer chunks keep the transform/evac pipeline smoothly fed), output
    fp16 in 4-block groups; all DMA partition lines are >= 512 B so they
    run at full modeled bandwidth.  Every load rides one queue (SP) in program
    order so the DMA device's FIFO sees a deterministic stream; y stores
    interleave [.., y(g-2), A(g+2), ..] so prefetched A acquires can
    never starve behind them.
  - Scheduling: dummy warm-up matmuls hold the PE p-state ramp while the
    first xt chunk lands; transform ib-pairs evacuate as single [128,1024]
    copies alternating DVE/ACT (ACT-biased); jb0/jb1's self matmuls are
    emitted up front (they only need xt chunk 0); the final block's
    store queues are arranged so no parked DMA config sits on the drain.
  - Per-core budget (cost model): PE = 16K (transform) + 32*(4*128 self +
    16*256 DR) = 164K cycles ~ 68 us; DMA = 16+4+4 MB ~ 70 us busy;
    span ~ 82 us = 16 us load-paced front + 61.5 us PE-floor chain stream
    + 4 us store/semaphore tail.
"""

import numpy as np

B, N, D = 128, 4096, 32
NCORES = 8
BSH = B // NCORES          # 16 batches per core
NIB = N // 128             # 32 node blocks of 128
NKT = N // 256             # 16 k-tiles of 256 for DoubleRow
NB4 = BSH // 4             # 4 groups of 4 batches
BQ = BSH * D               # 512 = (b, q) free width
GRP = 4                    # j-blocks per A-load / y-store group
NG = NIB // GRP            # 8 groups
SCALE = 4096.0             # 2^12 prescale on adj and W_self

_CACHE = {}


def _build_program():
    import concourse.bacc as bacc
    import concourse.mybir as mybir
    import concourse.tile as tile
    from contextlib import ExitStack

    f32 = mybir.dt.float32
    fp16 = mybir.dt.float16
    fp8 = mybir.dt.float8e4
    DR = mybir.MatmulPerfMode.DoubleRow

    nc = bacc.Bacc(
        trn_type="TRN2", target_bir_lowering=False, debug=False, num_devices=NCORES
    )
    xt = nc.dram_tensor("xt", [128, NIB * NB4 * 128], fp16, kind="ExternalInput").ap()
    bd = nc.dram_tensor("bd", [128, 256], fp16, kind="ExternalInput").ap()
    a = nc.dram_tensor("a", [128, NIB * NKT * 2 * 128], fp8, kind="ExternalInput").ap()
    y = nc.dram_tensor("y", [N, BQ], fp16, kind="ExternalOutput").ap()

    with tile.TileContext(nc) as tc, ExitStack() as ctx:
        const_pool = ctx.enter_context(tc.tile_pool(name="const", bufs=1))
        xt_pool = ctx.enter_context(tc.tile_pool(name="xtp", bufs=1))
        t_pool = ctx.enter_context(tc.tile_pool(name="tp", bufs=1))
        a_pool = ctx.enter_context(tc.tile_pool(name="ap", bufs=4))
        y_pool = ctx.enter_context(tc.tile_pool(name="yp", bufs=4))
        pt_pool = ctx.enter_context(tc.tile_pool(name="ptp", bufs=3, space="PSUM"))
        po_pool = ctx.enter_context(tc.tile_pool(name="pop", bufs=2, space="PSUM"))

        bd_sb = const_pool.tile([128, 256], fp16)

        # warm-up fodder: a zeroed SBUF tile and a throwaway PSUM pair-tile
        # for dummy matmuls that keep the PE "continuously executing" while
        # the xt chunks stream in, so the p-state ramp finishes early
        dum_sb = const_pool.tile([128, 512], fp16)
        nc.gpsimd.memset(dum_sb[:], 0)
        pd = pt_pool.tile([128, 2, BQ], f32, tag="pt", name="pd")

        def warmup(n):
            for _ in range(n):
                nc.tensor.matmul(
                    pd[:, 0, :], dum_sb[:, 0:128], dum_sb[:], start=True, stop=True
                )

        # xt resident: [p=(bh,pp), (ib, b4, il)]
        # All loads go through nc.sync in program order: the DMA device serves
        # acquires FIFO, so a single queue gives a deterministic stream order
        # (mixing engines lets prefetched A acquires cut ahead of late xt ones).
        xt_sb = xt_pool.tile([128, NIB, NB4, 128], fp16)
        xt_r = xt.rearrange("p (ib b4 il) -> p ib b4 il", ib=NIB, b4=NB4)
        # Small first chunk starts the transform/evac pipeline early; small
        # last chunk minimizes the T-tail the aggregation has to wait for.
        XT_CHUNKS = (2,) * 16
        xt_off = []
        o = 0
        for ln in XT_CHUNKS:
            xt_off.append((o, ln))
            o += ln

        # a is host-laid-out for DoubleRow: a[p, (jb, kt, two, jj)]
        a_r = a.rearrange(
            "p (jb kt two jj) -> p jb kt two jj", jb=NIB, kt=NKT, two=2
        )
        a_g0 = a_pool.tile([128, GRP, NKT, 2, 128], fp8, tag="a", name="a0")

        # stream order: c0 first (starts the transform pipeline), bd second
        # (first needed by the first matmul, ~2 us later), then the big xt
        # chunks; A panels follow — T completion is gated by the last xt
        # sliver, and jb0's first k-step starts after p0 lands either way.
        for ci, (o, ln) in enumerate(xt_off):
            nc.sync.dma_start(
                xt_sb[:, o : o + ln, :, :], xt_r[:, o : o + ln, :, :]
            )
            if ci == 0:
                nc.sync.dma_start(bd_sb[:], bd[:])
        # first panels in half-panel pieces: the chain stream starts as soon
        # as the transform drains instead of waiting a full panel + sem
        for s, (k0, k1) in ((0, (0, 8)), (0, (8, 16)), (1, (0, 8)), (1, (8, 16)),
                            (2, (0, 8)), (2, (8, 16))):
            nc.sync.dma_start(a_g0[:, s, k0:k1, :, :], a_r[:, s, k0:k1, :, :])
        nc.sync.dma_start(a_g0[:, 3:4, :, :, :], a_r[:, 3:4, :, :, :])

        # T (= X @ Wn, fp8): [p, (ib, b, q)]; ib-pairs feed DoubleRow k-tiles
        t_sb = t_pool.tile([128, NIB, BSH, D], fp8)

        # ---- transform: T = X @ Wn via block-diag weights ----
        # ib pairs share one 2-bank PSUM tile and evacuate in a single
        # [128, 1024] copy, halving per-instruction overhead; ACT gets the
        # larger share (it is faster per element than DVE here). The final
        # pair is split across both engines so the aggregation isn't gated
        # on one serial copy.
        # a few dummy matmuls ramp the PE p-state while the first chunk lands
        WARM = (8,) + (0,) * 15
        # jb0/jb1's self matmuls only need xt chunk 0 and bd: emit them right
        # after the first pair so those two chains are 0.2 us shorter when
        # the aggregation stream starts; their po banks are simply held open.
        po01 = [
            po_pool.tile([128, BQ], f32, tag="po", name=f"po{j}") for j in range(2)
        ]

        def early_self(j):
            for b4 in range(NB4):
                nc.tensor.matmul(
                    po01[j][:, b4 * 128 : (b4 + 1) * 128],
                    xt_sb[:, j, b4, :],
                    bd_sb[:, 128:256],
                    start=(b4 == 0),
                    stop=False,
                )

        ACT_PAIRS = {0, 2, 4, 6, 8, 9, 11, 13, 14}  # of 16 pairs; rest on DVE
        for ci, (o, ln) in enumerate(xt_off):
            warmup(WARM[ci])
            for pr in range(o // 2, (o + ln) // 2):
                pt = pt_pool.tile([128, 2, BQ], f32, tag="pt", name=f"pt{pr}")
                for h in range(2):
                    ib = 2 * pr + h
                    for b4 in range(NB4):
                        nc.tensor.matmul(
                            pt[:, h, b4 * 128 : (b4 + 1) * 128],
                            xt_sb[:, ib, b4, :],
                            bd_sb[:, 0:128],
                            start=(b4 == 0),
                            stop=(b4 == NB4 - 1),
                        )
                dst = t_sb[:, 2 * pr : 2 * pr + 2, :, :]
                src = pt.rearrange("p h (b q) -> p h b q", q=D)
                if pr == NIB // 2 - 1:
                    nc.vector.tensor_copy(dst[:, 0, :, :], src[:, 0, :, :])
                    nc.scalar.copy(dst[:, 1, :, :], src[:, 1, :, :])
                elif pr in ACT_PAIRS:
                    nc.scalar.copy(dst, src)
                else:
                    nc.vector.tensor_copy(dst, src)
                if ci == 0 and pr == 0:
                    early_self(0)
                    early_self(1)

        # y viewed as [g, p, jl, q] so grouped stores match the yg tile dims
        y_r = y.rearrange("(g jl p) q -> g p jl q", jl=GRP, p=128)

        # ---- aggregation: po[j, (b,q)] = X[j] @ (S*Ws)  +  sum_kt A2[kt]^T @ T[kt] ----
        # A tiles prefetch two groups ahead; completed y tiles store two
        # groups behind — all on the sync queue, so the DMA device sees one
        # deterministic stream [.., y(g-2), A(g+2), ..] and late y acquires
        # can never be starved behind prefetched A acquires.
        a_tiles = {0: a_g0}
        y_tiles = {}

        def a_prefetch(g):
            a_t = a_pool.tile([128, GRP, NKT, 2, 128], fp8, tag="a", name=f"a{g}")
            if g == 1:
                subs = ((0, 1), (1, 2), (2, 4))
            else:
                subs = ((0, 2), (2, 4))
            for s0, s1 in subs:
                nc.sync.dma_start(
                    a_t[:, s0:s1, :, :, :],
                    a_r[:, g * GRP + s0 : g * GRP + s1, :, :, :],
                )
            a_tiles[g] = a_t

        def y_flush(g):
            nc.sync.dma_start(y_r[g], y_tiles.pop(g)[:])

        a_prefetch(1)

        for g in range(NG):
            if g - 2 >= 0:
                y_flush(g - 2)
            if g + 2 < NG:
                a_prefetch(g + 2)
            if g == NG - 1:
                y_flush(NG - 2)
            a_t = a_tiles.pop(g)
            yg = y_pool.tile([128, GRP, BQ], fp16, tag="y", name=f"y{g}")
            y_tiles[g] = yg
            for jl in range(GRP):
                jb = g * GRP + jl
                if jb < 2:
                    po = po01[jb]  # self part already accumulated up front
                else:
                    po = po_pool.tile([128, BQ], f32, tag="po", name=f"po{jb}")
                    # self part: 4 fp16 matmuls, first starts the psum group
                    for b4 in range(NB4):
                        nc.tensor.matmul(
                            po[:, b4 * 128 : (b4 + 1) * 128],
                            xt_sb[:, jb, b4, :],
                            bd_sb[:, 128:256],
                            start=(b4 == 0),
                            stop=False,
                        )
                # neighbor part: 16 fp8 DoubleRow matmuls, 256-deep each
                for kt in range(NKT):
                    nc.tensor.matmul(
                        po[:],
                        a_t[:, jl, kt, :, :],
                        t_sb[:, 2 * kt : 2 * kt + 2, :, :],
                        start=False,
                        stop=(kt == NKT - 1),
                        perf_mode=DR,
                    )
                # relu + cast to fp16, alternate DVE / ACT; the final group
                # stores per-block, and jb31's store is issued on the same
                # engine (ACT) as its evac so no cross-engine semaphore hop
                # sits on the critical tail
                if jl % 2 == 0:
                    nc.vector.tensor_scalar_max(yg[:, jl, :], po[:], 0.0)
                else:
                    nc.scalar.activation(
                        yg[:, jl, :], po[:], mybir.ActivationFunctionType.Relu
                    )
                # final-group stores: jl2's rides the scalar queue so the SP
                # queue has no parked config in front of the very last store
                if g == NG - 1:
                    if jl == 1:
                        nc.sync.dma_start(y_r[g, :, 0:2, :], yg[:, 0:2, :])
                    elif jl == 2:
                        nc.scalar.dma_start(
                            y_r[g, :, jl : jl + 1, :], yg[:, jl : jl + 1, :]
                        )
                    elif jl == 3:
                        nc.sync.dma_start(
                            y_r[g, :, jl : jl + 1, :], yg[:, jl : jl + 1, :]
                        )
    nc.compile()
    return nc


def _get_program():
    if "nc" not in _CACHE:
        _CACHE["nc"] = _build_program()
    return _CACHE["nc"]


def _to_fp8(x):
    import ml_dtypes

    dt = getattr(ml_dtypes, "float8_e4m3", None) or ml_dtypes.float8_e4m3fn
    return x.astype(dt)


def make_in_maps(x3, adj, W_neigh, W_self):
    # bd: cols 0:128 block-diag 4x Wn; cols 128:256 block-diag 4x (SCALE*Ws)
    bd = np.zeros((128, 256), dtype=np.float32)
    for bh in range(4):
        bd[bh * 32 : (bh + 1) * 32, bh * 32 : bh * 32 + 32] = W_neigh
        bd[bh * 32 : (bh + 1) * 32, 128 + bh * 32 : 128 + bh * 32 + 32] = (
            W_self * SCALE
        )
    bd = bd.astype(np.float16)

    # a2[p, (jb, kt, two, jj)] = SCALE * adj[kt*256 + two*128 + p, jb*128 + jj]
    a2 = _to_fp8(
        np.ascontiguousarray(
            (adj * np.float32(SCALE))
            .reshape(NKT, 2, 128, NIB, 128)
            .transpose(2, 3, 0, 1, 4)
        ).reshape(128, NIB * NKT * 2 * 128)
    )

    in_maps = []
    for c in range(NCORES):
        xs = x3[c * BSH : (c + 1) * BSH]  # [16, N, 32]
        # XT[(bh*32+p), (ib, b4, il)] = xs[b4*4 + bh, ib*128 + il, p]
        xtc = np.ascontiguousarray(
            xs.reshape(NB4, 4, NIB, 128, D).transpose(1, 4, 2, 0, 3)
        ).reshape(128, NB4 * N).astype(np.float16)
        in_maps.append({"xt": xtc, "bd": bd, "a": a2})
    return in_maps


def kernel(inputs, adj, W_neigh, W_self, batch_train=None):
    from concourse.bass_utils import run_bass_kernel_spmd

    inputs = np.asarray(inputs, dtype=np.float32)
    adj = np.ascontiguousarray(np.asarray(adj, dtype=np.float32))
    W_neigh = np.asarray(W_neigh, dtype=np.float32)
    W_self = np.asarray(W_self, dtype=np.float32)

    x3 = inputs.reshape(B, N, D)
    in_maps = make_in_maps(x3, adj, W_neigh, W_self)

    nc = _get_program()
    res = run_bass_kernel_spmd(nc, in_maps, list(range(NCORES)))

    inv = np.float32(1.0 / SCALE)
    out = np.empty((B, N * D), dtype=np.float32)
    for c in range(NCORES):
        yc = np.asarray(res.results[c]["y"], dtype=np.float32)  # [N, (b, q)]
        out[c * BSH : (c + 1) * BSH] = (
            yc.reshape(N, BSH, D).transpose(1, 0, 2).reshape(BSH, N * D) * inv
        )
    return out


# revision 80
# speedup vs baseline: 1.0063x; 1.0063x over previous
"""GraphSAGE layer kernel for Trainium2, SPMD over 8 NeuronCores.

Math (per reference):
    x3   = inputs.reshape(B, N, D)                      # B=128, N=4096, D=32
    out  = relu(x3 @ W_self + (A^T_contract @ (x3 @ W_neigh)))   # per batch
    out  = out.reshape(B, N*D)
with out[b, j, q] = relu(sum_p x3[b,j,p] Ws[p,q] + sum_i adj[i,j] (x3 @ Wn)[b,i,q])

Strategy:
  - Pure data-parallel over batch: 16 batches per core.
  - The big aggregation matmul runs in fp8 (e4m3) with DoubleRow perf mode:
    each matmul consumes a 256-deep contraction (2 k-tiles of 128) at half
    the per-row cost of fp16 -> 4x fewer PE cycles than the fp16 baseline.
    adj is host-prescaled by 2^12 so its entries (~1/N) sit in the fp8
    normal range; W_self is prescaled by 2^12 to match, and the host
    multiplies the output by 2^-12 after gather (relu commutes with
    positive scaling).
  - T = X @ Wn is computed on-device in fp16 (via a block-diagonal weight
    trick: 4 batches x 32 features on the 128 contraction lanes), then cast
    to fp8 during PSUM evacuation.  The self part X @ (2^12 Ws) is NOT
    materialized: it is accumulated directly into the aggregation PSUM with
    4 extra fp16 matmuls per output block (stationary = resident XT chunk).
  - Precision: X and Ws stay fp16 (the self term dominates the output).
    Only adj and T are fp8; they only feed the neighbor term whose
    magnitude is ~1/sqrt(N) of the self term, so fp8 error there is ~1e-3
    relative on the output, far below the 2e-2 gate.
  - DMA: A is fp8 (16 MB/core: half-panel pieces for the first three
    panels, then 1/2-panel sub-groups), XT fp16 in 16 two-block chunks
    (finer chunks keep the transform/evac pipeline smoothly fed), output
    fp16 in 4-block groups; all DMA partition lines are >= 512 B so they
    run at full modeled bandwidth.  Every load rides one queue (SP) in program
    order so the DMA device's FIFO sees a deterministic stream; y stores
    interleave [.., y(g-2), A(g+2), ..] so prefetched A acquires can
    never starve behind them.
  - Scheduling: dummy warm-up matmuls hold the PE p-state ramp while the
    first xt chunk lands; transform ib-pairs evacuate as single [128,1024]
    copies alternating DVE/ACT (ACT-biased); jb0/jb1's self matmuls are
    emitted up front (they only need xt chunk 0); the final block's
    store queues are arranged so no parked DMA config sits on the drain.
  - Per-core budget (cost model): PE = 16K (transform) + 32*(4*128 self +
    16*256 DR) = 164K cycles ~ 68 us; DMA = 16+4+4 MB ~ 70 us busy;
    span ~ 81 us = 15.5 us load-paced front + 61.5 us PE-floor chain stream
    + 4 us store/semaphore tail.
"""

import numpy as np

B, N, D = 128, 4096, 32
NCORES = 8
BSH = B // NCORES          # 16 batches per core
NIB = N // 128             # 32 node blocks of 128
NKT = N // 256             # 16 k-tiles of 256 for DoubleRow
NB4 = BSH // 4             # 4 groups of 4 batches
BQ = BSH * D               # 512 = (b, q) free width
GRP = 4                    # j-blocks per A-load / y-store group
NG = NIB // GRP            # 8 groups
SCALE = 4096.0             # 2^12 prescale on adj and W_self

_CACHE = {}


def _build_program():
    import concourse.bacc as bacc
    import concourse.mybir as mybir
    import concourse.tile as tile
    from contextlib import ExitStack

    f32 = mybir.dt.float32
    fp16 = mybir.dt.float16
    fp8 = mybir.dt.float8e4
    DR = mybir.MatmulPerfMode.DoubleRow

    nc = bacc.Bacc(
        trn_type="TRN2", target_bir_lowering=False, debug=False, num_devices=NCORES
    )
    xt = nc.dram_tensor("xt", [128, NIB * NB4 * 128], fp16, kind="ExternalInput").ap()
    bd = nc.dram_tensor("bd", [128, 256], fp16, kind="ExternalInput").ap()
    a = nc.dram_tensor("a", [128, NIB * NKT * 2 * 128], fp8, kind="ExternalInput").ap()
    y = nc.dram_tensor("y", [N, BQ], fp16, kind="ExternalOutput").ap()

    with tile.TileContext(nc) as tc, ExitStack() as ctx:
        const_pool = ctx.enter_context(tc.tile_pool(name="const", bufs=1))
        xt_pool = ctx.enter_context(tc.tile_pool(name="xtp", bufs=1))
        t_pool = ctx.enter_context(tc.tile_pool(name="tp", bufs=1))
        a_pool = ctx.enter_context(tc.tile_pool(name="ap", bufs=4))
        y_pool = ctx.enter_context(tc.tile_pool(name="yp", bufs=4))
        pt_pool = ctx.enter_context(tc.tile_pool(name="ptp", bufs=3, space="PSUM"))
        po_pool = ctx.enter_context(tc.tile_pool(name="pop", bufs=2, space="PSUM"))

        bd_sb = const_pool.tile([128, 256], fp16)

        # warm-up fodder: a zeroed SBUF tile and a throwaway PSUM pair-tile
        # for dummy matmuls that keep the PE "continuously executing" while
        # the xt chunks stream in, so the p-state ramp finishes early
        dum_sb = const_pool.tile([128, 512], fp16)
        nc.gpsimd.memset(dum_sb[:], 0)
        pd = pt_pool.tile([128, 2, BQ], f32, tag="pt", name="pd")

        def warmup(n):
            for _ in range(n):
                nc.tensor.matmul(
                    pd[:, 0, :], dum_sb[:, 0:128], dum_sb[:], start=True, stop=True
                )

        # xt resident: [p=(bh,pp), (ib, b4, il)]
        # All loads go through nc.sync in program order: the DMA device serves
        # acquires FIFO, so a single queue gives a deterministic stream order
        # (mixing engines lets prefetched A acquires cut ahead of late xt ones).
        xt_sb = xt_pool.tile([128, NIB, NB4, 128], fp16)
        xt_r = xt.rearrange("p (ib b4 il) -> p ib b4 il", ib=NIB, b4=NB4)
        # Small first chunk starts the transform/evac pipeline early; small
        # last chunk minimizes the T-tail the aggregation has to wait for.
        XT_CHUNKS = (2,) * 16
        xt_off = []
        o = 0
        for ln in XT_CHUNKS:
            xt_off.append((o, ln))
            o += ln

        # a is host-laid-out for DoubleRow: a[p, (jb, kt, two, jj)]
        a_r = a.rearrange(
            "p (jb kt two jj) -> p jb kt two jj", jb=NIB, kt=NKT, two=2
        )
        a_g0 = a_pool.tile([128, GRP, NKT, 2, 128], fp8, tag="a", name="a0")

        # stream order: c0 first (starts the transform pipeline), bd second
        # (first needed by the first matmul, ~2 us later), then the big xt
        # chunks; A panels follow — T completion is gated by the last xt
        # sliver, and jb0's first k-step starts after p0 lands either way.
        for ci, (o, ln) in enumerate(xt_off):
            nc.sync.dma_start(
                xt_sb[:, o : o + ln, :, :], xt_r[:, o : o + ln, :, :]
            )
            if ci == 0:
                nc.sync.dma_start(bd_sb[:], bd[:])
        # first panels in half-panel pieces: the chain stream starts as soon
        # as the transform drains instead of waiting a full panel + sem
        for s, (k0, k1) in ((0, (0, 8)), (0, (8, 16)), (1, (0, 8)), (1, (8, 16)),
                            (2, (0, 8)), (2, (8, 16))):
            nc.sync.dma_start(a_g0[:, s, k0:k1, :, :], a_r[:, s, k0:k1, :, :])
        nc.sync.dma_start(a_g0[:, 3:4, :, :, :], a_r[:, 3:4, :, :, :])

        # T (= X @ Wn, fp8): [p, (ib, b, q)]; ib-pairs feed DoubleRow k-tiles
        t_sb = t_pool.tile([128, NIB, BSH, D], fp8)

        # ---- transform: T = X @ Wn via block-diag weights ----
        # ib pairs share one 2-bank PSUM tile and evacuate in a single
        # [128, 1024] copy, halving per-instruction overhead; ACT gets the
        # larger share (it is faster per element than DVE here). The final
        # pair is split across both engines so the aggregation isn't gated
        # on one serial copy.
        # a few dummy matmuls ramp the PE p-state while the first chunk lands
        WARM = (4,) + (0,) * 15
        # jb0/jb1's self matmuls only need xt chunk 0 and bd: emit them right
        # after the first pair so those two chains are 0.2 us shorter when
        # the aggregation stream starts; their po banks are simply held open.
        po01 = [
            po_pool.tile([128, BQ], f32, tag="po", name=f"po{j}") for j in range(2)
        ]

        def early_self(j):
            for b4 in range(NB4):
                nc.tensor.matmul(
                    po01[j][:, b4 * 128 : (b4 + 1) * 128],
                    xt_sb[:, j, b4, :],
                    bd_sb[:, 128:256],
                    start=(b4 == 0),
                    stop=False,
                )

        ACT_PAIRS = {0, 2, 4, 6, 8, 9, 11, 13, 14}  # of 16 pairs; rest on DVE
        for ci, (o, ln) in enumerate(xt_off):
            warmup(WARM[ci])
            for pr in range(o // 2, (o + ln) // 2):
                pt = pt_pool.tile([128, 2, BQ], f32, tag="pt", name=f"pt{pr}")
                for h in range(2):
                    ib = 2 * pr + h
                    for b4 in range(NB4):
                        nc.tensor.matmul(
                            pt[:, h, b4 * 128 : (b4 + 1) * 128],
                            xt_sb[:, ib, b4, :],
                            bd_sb[:, 0:128],
                            start=(b4 == 0),
                            stop=(b4 == NB4 - 1),
                        )
                dst = t_sb[:, 2 * pr : 2 * pr + 2, :, :]
                src = pt.rearrange("p h (b q) -> p h b q", q=D)
                if pr == NIB // 2 - 1:
                    nc.vector.tensor_copy(dst[:, 0, :, :], src[:, 0, :, :])
                    nc.scalar.copy(dst[:, 1, :, :], src[:, 1, :, :])
                elif pr in ACT_PAIRS:
                    nc.scalar.copy(dst, src)
                else:
                    nc.vector.tensor_copy(dst, src)
                if ci == 0 and pr == 0:
                    early_self(0)
                    early_self(1)

        # y viewed as [g, p, jl, q] so grouped stores match the yg tile dims
        y_r = y.rearrange("(g jl p) q -> g p jl q", jl=GRP, p=128)

        # ---- aggregation: po[j, (b,q)] = X[j] @ (S*Ws)  +  sum_kt A2[kt]^T @ T[kt] ----
        # A tiles prefetch two groups ahead; completed y tiles store two
        # groups behind — all on the sync queue, so the DMA device sees one
        # deterministic stream [.., y(g-2), A(g+2), ..] and late y acquires
        # can never be starved behind prefetched A acquires.
        a_tiles = {0: a_g0}
        y_tiles = {}

        def a_prefetch(g):
            a_t = a_pool.tile([128, GRP, NKT, 2, 128], fp8, tag="a", name=f"a{g}")
            if g == 1:
                subs = ((0, 1), (1, 2), (2, 4))
            else:
                subs = ((0, 2), (2, 4))
            for s0, s1 in subs:
                nc.sync.dma_start(
                    a_t[:, s0:s1, :, :, :],
                    a_r[:, g * GRP + s0 : g * GRP + s1, :, :, :],
                )
            a_tiles[g] = a_t

        def y_flush(g):
            nc.sync.dma_start(y_r[g], y_tiles.pop(g)[:])

        a_prefetch(1)

        for g in range(NG):
            if g - 2 >= 0:
                y_flush(g - 2)
            if g + 2 < NG:
                a_prefetch(g + 2)
            if g == NG - 1:
                y_flush(NG - 2)
            a_t = a_tiles.pop(g)
            yg = y_pool.tile([128, GRP, BQ], fp16, tag="y", name=f"y{g}")
            y_tiles[g] = yg
            for jl in range(GRP):
                jb = g * GRP + jl
                if jb < 2:
                    po = po01[jb]  # self part already accumulated up front
                else:
                    po = po_pool.tile([128, BQ], f32, tag="po", name=f"po{jb}")
                    # self part: 4 fp16 matmuls, first starts the psum group
                    for b4 in range(NB4):
                        nc.tensor.matmul(
                            po[:, b4 * 128 : (b4 + 1) * 128],
                            xt_sb[:, jb, b4, :],
                            bd_sb[:, 128:256],
                            start=(b4 == 0),
                            stop=False,
                        )
                # neighbor part: 16 fp8 DoubleRow matmuls, 256-deep each
                for kt in range(NKT):
                    nc.tensor.matmul(
                        po[:],
                        a_t[:, jl, kt, :, :],
                        t_sb[:, 2 * kt : 2 * kt + 2, :, :],
                        start=False,
                        stop=(kt == NKT - 1),
                        perf_mode=DR,
                    )
                # relu + cast to fp16, alternate DVE / ACT; the final group
                # stores per-block, and jb31's store is issued on the same
                # engine (ACT) as its evac so no cross-engine semaphore hop
                # sits on the critical tail
                if jl % 2 == 0:
                    nc.vector.tensor_scalar_max(yg[:, jl, :], po[:], 0.0)
                else:
                    nc.scalar.activation(
                        yg[:, jl, :], po[:], mybir.ActivationFunctionType.Relu
                    )
                # final-group stores: jl2's rides the scalar queue so the SP
                # queue has no parked config in front of the very last store
                if g == NG - 1:
                    if jl == 1:
                        nc.sync.dma_start(y_r[g, :, 0:2, :], yg[:, 0:2, :])
                    elif jl == 2:
                        nc.scalar.dma_start(
                            y_r[g, :, jl : jl + 1, :], yg[:, jl : jl + 1, :]
                        )
                    elif jl == 3:
                        nc.sync.dma_start(
                            y_r[g, :, jl : jl + 1, :], yg[:, jl : jl + 1, :]
                        )
    nc.compile()
    return nc


def _get_program():
    if "nc" not in _CACHE:
        _CACHE["nc"] = _build_program()
    return _CACHE["nc"]


def _to_fp8(x):
    import ml_dtypes

    dt = getattr(ml_dtypes, "float8_e4m3", None) or ml_dtypes.float8_e4m3fn
    return x.astype(dt)


def make_in_maps(x3, adj, W_neigh, W_self):
    # bd: cols 0:128 block-diag 4x Wn; cols 128:256 block-diag 4x (SCALE*Ws)
    bd = np.zeros((128, 256), dtype=np.float32)
    for bh in range(4):
        bd[bh * 32 : (bh + 1) * 32, bh * 32 : bh * 32 + 32] = W_neigh
        bd[bh * 32 : (bh + 1) * 32, 128 + bh * 32 : 128 + bh * 32 + 32] = (
            W_self * SCALE
        )
    bd = bd.astype(np.float16)

    # a2[p, (jb, kt, two, jj)] = SCALE * adj[kt*256 + two*128 + p, jb*128 + jj]
    a2 = _to_fp8(
        np.ascontiguousarray(
            (adj * np.float32(SCALE))
            .reshape(NKT, 2, 128, NIB, 128)
            .transpose(2, 3, 0, 1, 4)
        ).reshape(128, NIB * NKT * 2 * 128)
    )

    in_maps = []
    for c in range(NCORES):
        xs = x3[c * BSH : (c + 1) * BSH]  # [16, N, 32]
        # XT[(bh*32+p), (ib, b4, il)] = xs[b4*4 + bh, ib*128 + il, p]
        xtc = np.ascontiguousarray(
            xs.reshape(NB4, 4, NIB, 128, D).transpose(1, 4, 2, 0, 3)
        ).reshape(128, NB4 * N).astype(np.float16)
        in_maps.append({"xt": xtc, "bd": bd, "a": a2})
    return in_maps


def kernel(inputs, adj, W_neigh, W_self, batch_train=None):
    from concourse.bass_utils import run_bass_kernel_spmd

    inputs = np.asarray(inputs, dtype=np.float32)
    adj = np.ascontiguousarray(np.asarray(adj, dtype=np.float32))
    W_neigh = np.asarray(W_neigh, dtype=np.float32)
    W_self = np.asarray(W_self, dtype=np.float32)

    x3 = inputs.reshape(B, N, D)
    in_maps = make_in_maps(x3, adj, W_neigh, W_self)

    nc = _get_program()
    res = run_bass_kernel_spmd(nc, in_maps, list(range(NCORES)))

    inv = np.float32(1.0 / SCALE)
    out = np.empty((B, N * D), dtype=np.float32)
    for c in range(NCORES):
        yc = np.asarray(res.results[c]["y"], dtype=np.float32)  # [N, (b, q)]
        out[c * BSH : (c + 1) * BSH] = (
            yc.reshape(N, BSH, D).transpose(1, 0, 2).reshape(BSH, N * D) * inv
        )
    return out
